# revision 1
# baseline (speedup 1.0000x reference)
"""LoRA q/v + full self-attention (B=4, T=2048, H=768, R=64) on 8 TRN2 cores.

Sharding: data-parallel over batch x sequence-halves. Core c handles batch
b = c//2, query rows t0 = (c%2)*1024 .. +1024. Each core gets its batch's
full x (for k/v) plus LoRA weights (replicated). Host pre-transposes x[b]
to xT [H, T] and *rolls* the sequence axis by -t0 so the core's own query
rows are always columns 0:1024 of xT -- the device program is identical
across cores (SPMD), only the data differs. Attention is order-invariant
over the key axis, so the roll only changes fp summation order.

Device kernel (per core):
  uqT = Aq^T @ xT[:, :1024]; qT = xT[:, :1024] + Bq^T @ uqT     (LoRA q)
  uvT = Av^T @ xT;          v  = xn + (Bv^T @ uvT)^T            (LoRA v)
  v is stored [s, 0:768] with col 768 = 1.0 (ones column).
  scoresT[s, t] = sum_h xT[h, s] * qT[h, t]    (PE, PSUM accum over 6 h-chunks)
  attT = exp(scoresT * scale + bias[s])        (ACT; bias = 0 or -1e30 from mask;
                                                no max-subtraction: |scores*scale| ~ 5)
  outp[t, 0:769] = sum_s attT[s, t-chunk] * v[s, :]  (PE; col 768 = softmax denom)
  out[t, :] = outp[t, 0:768] * (1 / outp[t, 768])    (DVE drain, fused normalize)
"""

import numpy as np


def _ensure_path():
    try:
        import concourse  # noqa: F401
    except ImportError:
        import sys

        for p in ("/opt/trn_rl_repo", "/root/.axon_site/_ro/trn_rl_repo"):
            sys.path.insert(0, p)
            try:
                import concourse  # noqa: F401

                return
            except ImportError:
                sys.path.pop(0)
        raise


_ensure_path()

import concourse.bass as bass  # noqa: E402
from concourse import bacc  # noqa: E402
import concourse.tile as tile  # noqa: E402
from concourse import mybir  # noqa: E402
from concourse.bass_utils import run_bass_kernel_spmd  # noqa: E402
from concourse.vector_clock import ScopedClock, VectorClock  # noqa: E402


# --- workaround: this walrus build rejects >1 sync-wait on the TileContext
# kernel-tail drain ("Too many sync wait commands", CoreV3GenImpl.cpp:104).
# Emit one drain per busy proc, each carrying a single sem wait.
def _patched_drain_and_barrier(self, tick_clock, wait_clock):
    gc = tick_clock.global_clock
    n = len(gc)
    for p in range(n):
        t = gc[p]
        if t <= 0:
            continue
        vec = [0] * n
        vec[p] = t
        d = self.nc.sync.drain()
        wait_clock.add_sem_waits(d.ins, ScopedClock({None: VectorClock(vec)}))

    self.nc.all_engine_barrier()
    assert self.sems is not None
    popped = self.nc._tile_sem_poison_stack.pop()
    assert popped is self._sem_poison
    self.nc.clear_and_free_semaphores(list(self.sems.allocated().values()))
    self.nc.all_engine_barrier()


tile.TileContext._drain_and_barrier = _patched_drain_and_barrier

B, T, H, R = 4, 2048, 768, 64
HC = H // 128  # 6 h-chunks
SC = T // 128  # 16 s-chunks
TQ = T // 2  # 1024 query rows per core
SCALE = float(1.0 / np.sqrt(H))
FP32 = mybir.dt.float32
I32 = mybir.dt.int32
Exp = mybir.ActivationFunctionType.Exp
ALU = mybir.AluOpType

LAST_RESULTS = None  # BassKernelResults of the most recent run (for profiling)


def _emit(tc, nc, xT, xn, aq, bq, av, bv, mk, out):
    from contextlib import ExitStack

    with ExitStack() as ctx:
        p_xT = ctx.enter_context(tc.tile_pool(name="p_xT", bufs=1))
        p_q = ctx.enter_context(tc.tile_pool(name="p_q", bufs=1))
        p_v = ctx.enter_context(tc.tile_pool(name="p_v", bufs=1))
        p_att = ctx.enter_context(tc.tile_pool(name="p_att", bufs=1))
        p_xn = ctx.enter_context(tc.tile_pool(name="p_xn", bufs=3))
        p_w = ctx.enter_context(tc.tile_pool(name="p_w", bufs=1))
        p_u = ctx.enter_context(tc.tile_pool(name="p_u", bufs=1))
        p_o = ctx.enter_context(tc.tile_pool(name="p_o", bufs=3))
        p_r = ctx.enter_context(tc.tile_pool(name="p_r", bufs=4))

        # ---- weights / mask bias (all DMAs rows-contiguous: this walrus
        # build rejects sync-waits on strided DIRECT2D pseudo-DMAs) ----
        aq_sb = [p_w.tile([128, R], FP32, name=f"aq_sb{i}") for i in range(HC)]
        av_sb = [p_w.tile([128, R], FP32, name=f"av_sb{i}") for i in range(HC)]
        for i in range(HC):
            nc.gpsimd.dma_start(out=aq_sb[i][:, :], in_=aq[i * 128 : (i + 1) * 128, :])
            nc.gpsimd.dma_start(out=av_sb[i][:, :], in_=av[i * 128 : (i + 1) * 128, :])
        bq_sb = p_w.tile([R, H], FP32, name="bq_sb")
        bv_sb = p_w.tile([R, H], FP32, name="bv_sb")
        nc.gpsimd.dma_start(out=bq_sb[:, :], in_=bq[:, :])
        nc.gpsimd.dma_start(out=bv_sb[:, :], in_=bv[:, :])

        # bias[s] = (mask-1)*1e30, precomputed host-side, one [128,1] per s-chunk
        bias_t = [p_w.tile([128, 1], FP32, name=f"bias{j}") for j in range(SC)]
        for j in range(SC):
            nc.gpsimd.dma_start(out=bias_t[j][:, :], in_=mk[j : j + 1, :].rearrange("n p -> p n"))

        xT_sb = [p_xT.tile([128, T], FP32, name=f"xT{i}") for i in range(HC)]
        for i in range(HC):
            nc.gpsimd.dma_start(out=xT_sb[i][:, :], in_=xT[i * 128 : (i + 1) * 128, :])

        q_sb = [p_q.tile([128, TQ], FP32, name=f"q{i}") for i in range(HC)]
        uq_sb = p_u.tile([R, TQ], FP32, name="uq_sb")
        uv_sb = p_u.tile([R, T], FP32, name="uv_sb")

        with tc.tile_pool(name="psL", bufs=2, space="PSUM") as psL:
            # uqT [64, TQ] = Aq^T @ xT[:, :TQ]
            for tq in range(TQ // 512):
                ps = psL.tile([64, 512], FP32, name="psl", tag="psl")
                for i in range(HC):
                    nc.tensor.matmul(
                        ps[:, :],
                        lhsT=aq_sb[i][:, :],
                        rhs=xT_sb[i][:, tq * 512 : (tq + 1) * 512],
                        start=(i == 0),
                        stop=(i == HC - 1),
                    )
                nc.scalar.copy(uq_sb[:, tq * 512 : (tq + 1) * 512], ps[:, :])
            # qT = xT[:, :TQ] + Bq^T @ uqT
            for i in range(HC):
                for tq in range(TQ // 512):
                    ps = psL.tile([128, 512], FP32, name="pslq", tag="psl")
                    nc.tensor.matmul(
                        ps[:, :],
                        lhsT=bq_sb[:, i * 128 : (i + 1) * 128],
                        rhs=uq_sb[:, tq * 512 : (tq + 1) * 512],
                        start=True,
                        stop=True,
                    )
                    nc.vector.tensor_add(
                        q_sb[i][:, tq * 512 : (tq + 1) * 512],
                        ps[:, :],
                        xT_sb[i][:, tq * 512 : (tq + 1) * 512],
                    )
            # uvT [64, T] = Av^T @ xT
            for sk in range(T // 512):
                ps = psL.tile([64, 512], FP32, name="pslv", tag="psl")
                for i in range(HC):
                    nc.tensor.matmul(
                        ps[:, :],
                        lhsT=av_sb[i][:, :],
                        rhs=xT_sb[i][:, sk * 512 : (sk + 1) * 512],
                        start=(i == 0),
                        stop=(i == HC - 1),
                    )
                nc.scalar.copy(uv_sb[:, sk * 512 : (sk + 1) * 512], ps[:, :])
            # v[s, :768] = xn[s, :] + (Bv^T @ uvT)^T ; v[s, 768] = 1.0
            v_sb = []
            for j in range(SC):
                xnt = p_xn.tile([128, H], FP32, name="xnt")
                nc.gpsimd.dma_start(out=xnt[:, :], in_=xn[j * 128 : (j + 1) * 128, :])
                vj = p_v.tile([128, 772], FP32, name=f"v{j}")
                nc.vector.memset(vj[:, 768:769], 1.0)
                ps = psL.tile([128, 768], FP32, name="pslc", tag="psl")
                nc.tensor.matmul(
                    ps[:, 0:512],
                    lhsT=uv_sb[:, j * 128 : (j + 1) * 128],
                    rhs=bv_sb[:, 0:512],
                    start=True,
                    stop=True,
                )
                nc.tensor.matmul(
                    ps[:, 512:768],
                    lhsT=uv_sb[:, j * 128 : (j + 1) * 128],
                    rhs=bv_sb[:, 512:768],
                    start=True,
                    stop=True,
                )
                nc.vector.tensor_add(vj[:, 0:768], ps[:, 0:768], xnt[:, :])
                v_sb.append(vj)

        # ---- attention: 2 superblocks of 512 query cols ----
        with (
            tc.tile_pool(name="ps_s", bufs=2, space="PSUM") as ps_s,
            tc.tile_pool(name="ps_o", bufs=3, space="PSUM") as ps_o,
        ):
            for SB in range(2):
                att = []
                for j in range(SC):
                    ps = ps_s.tile([128, 512], FP32, name="pss", tag="pss")
                    for i in range(HC):
                        nc.tensor.matmul(
                            ps[:, :],
                            lhsT=xT_sb[i][:, j * 128 : (j + 1) * 128],
                            rhs=q_sb[i][:, SB * 512 : (SB + 1) * 512],
                            start=(i == 0),
                            stop=(i == HC - 1),
                        )
                    attj = p_att.tile([128, 512], FP32, name=f"att{j}")
                    nc.scalar.activation(
                        attj[:, :], ps[:, :], Exp, bias=bias_t[j][:, :], scale=SCALE
                    )
                    att.append(attj)
                for pair in range(2):
                    pso = [
                        ps_o.tile([128, 772], FP32, name="pso", tag="pso") for _ in range(2)
                    ]
                    for j in range(SC):
                        for c in range(2):
                            lc = pair * 2 + c
                            nc.tensor.matmul(
                                pso[c][:, 0:512],
                                lhsT=att[j][:, lc * 128 : (lc + 1) * 128],
                                rhs=v_sb[j][:, 0:512],
                                start=(j == 0),
                                stop=(j == SC - 1),
                            )
                            nc.tensor.matmul(
                                pso[c][:, 512:769],
                                lhsT=att[j][:, lc * 128 : (lc + 1) * 128],
                                rhs=v_sb[j][:, 512:769],
                                start=(j == 0),
                                stop=(j == SC - 1),
                            )
                    for c in range(2):
                        lc = pair * 2 + c
                        tr = SB * 512 + lc * 128
                        rc = p_r.tile([128, 1], FP32, name="rc")
                        nc.vector.reciprocal(rc[:, :], pso[c][:, 768:769])
                        ob = p_o.tile([128, H], FP32, name="ob")
                        nc.vector.tensor_scalar(
                            ob[:, :], pso[c][:, 0:768], rc[:, :], None, ALU.mult
                        )
                        nc.gpsimd.dma_start(out=out[tr : tr + 128, :], in_=ob[:, :])


_NC_CACHE = None


def _build_nc():
    global _NC_CACHE
    if _NC_CACHE is not None:
        return _NC_CACHE
    nc = bacc.Bacc("TRN2", target_bir_lowering=False, debug=False)
    xT = nc.dram_tensor("xT", [H, T], FP32, kind="ExternalInput").ap()
    xn = nc.dram_tensor("xn", [T, H], FP32, kind="ExternalInput").ap()
    aq = nc.dram_tensor("aq", [H, R], FP32, kind="ExternalInput").ap()
    bq = nc.dram_tensor("bq", [R, H], FP32, kind="ExternalInput").ap()
    av = nc.dram_tensor("av", [H, R], FP32, kind="ExternalInput").ap()
    bv = nc.dram_tensor("bv", [R, H], FP32, kind="ExternalInput").ap()
    mk = nc.dram_tensor("mk", [SC, 128], FP32, kind="ExternalInput").ap()
    out = nc.dram_tensor("out", [TQ, H], FP32, kind="ExternalOutput").ap()

    import os

    linearize = bool(int(os.environ.get("KERNEL_LINEARIZE", "0")))
    with tile.TileContext(nc, linearize=linearize) as tc:
        _emit(tc, nc, xT, xn, aq, bq, av, bv, mk, out)
    nc.compile()
    _NC_CACHE = nc
    return nc


def kernel(hidden_states, mask, A_q, B_q, A_v, B_v):
    global LAST_RESULTS
    import os

    x = np.asarray(hidden_states, dtype=np.float32)
    mask = np.asarray(mask, dtype=np.int32)
    A_q = np.ascontiguousarray(A_q, dtype=np.float32)
    B_q = np.ascontiguousarray(B_q, dtype=np.float32)
    A_v = np.ascontiguousarray(A_v, dtype=np.float32)
    B_v = np.ascontiguousarray(B_v, dtype=np.float32)

    in_maps = []
    for c in range(8):
        b, t0 = c // 2, (c % 2) * TQ
        xr = np.roll(x[b], -t0, axis=0)  # [T, H], core's q rows first
        in_maps.append(
            {
                "xT": np.ascontiguousarray(xr.T),
                "xn": np.ascontiguousarray(xr),
                "aq": A_q,
                "bq": B_q,
                "av": A_v,
                "bv": B_v,
                "mk": ((np.roll(mask[b], -t0).reshape(SC, 128).astype(np.float32)) - 1.0) * 1e30,
            }
        )

    nc = _build_nc()
    trace = bool(int(os.environ.get("KERNEL_TRACE", "0")))
    res = run_bass_kernel_spmd(nc, in_maps, core_ids=list(range(8)), trace=trace)
    LAST_RESULTS = res

    outp = np.empty((B, T, H), dtype=np.float32)
    for c in range(8):
        b, t0 = c // 2, (c % 2) * TQ
        outp[b, t0 : t0 + TQ] = res.results[c]["out"]
    return outp



# revision 5
# speedup vs baseline: 5.8844x; 5.8844x over previous
"""LoRA q/v + full self-attention (B=4, T=2048, H=768, R=64) on TRN2.

The wall-clock of a call in this environment is dominated by the axon
relay wire (~65 MB/s) and per-dispatch latency (~70 ms), not device
compute (~0.5 ms). So the design minimizes bytes on the wire and host
work, and caches the jitted executable across calls:

  - 4 cores, one full batch each (cores 4-7 unused). x is shipped
    exactly once as a zero-copy [B*T, H] view sharded over the 4 cores
    -- no per-core duplication, no host-side transpose/roll.
  - bf16 on the wire both directions (x in, out back): ~25 MB total
    per call vs ~125 MB for the fp32 data-parallel-with-duplication
    layout. bf16 also gives 4x PE throughput on device.
  - The jax.jit(shard_map(bass_exec)) callable is built ONCE and
    reused; run_bass_kernel_spmd rebuilds (and re-compiles) it every
    call, which costs seconds per call.
  - Output buffers are donated zeros created on-device (jitted zeros
    fn) instead of shipping 12 MB of host zeros.

Device kernel (per core, batch b = core id, all of T=2048 as queries):
  xT = transpose(x) on device via PE (96 128x128 transposes)
  uqT = Aq^T @ xT; qT = xT + Bq^T @ uqT                   (LoRA q)
  uvT = Av^T @ xT; v  = x + (Bv^T @ uvT)^T               (LoRA v)
  v stored [s, 0:768] with col 768 = 1.0 (ones column).
  per 512-wide query superblock SB (4 of them):
    scoresT[s, t] = sum_h xT[h, s] * qT[h, t]   (PE, PSUM over 6 h-chunks)
    attT = exp(scoresT * scale + bias[s])       (ACT; bias = 0 or -1e30
                                                 from mask; no max-sub:
                                                 |scores*scale| ~ 5)
    outp[t, 0:769] = sum_s attT[s, t'] * v[s, :]  (PE; col 768 = denom)
    out[t, :] = outp[t, 0:768] * (1/outp[t, 768]) (DVE, written bf16)
"""

import numpy as np


def _ensure_path():
    try:
        import concourse  # noqa: F401
    except ImportError:
        import sys

        for p in ("/opt/trn_rl_repo", "/root/.axon_site/_ro/trn_rl_repo"):
            sys.path.insert(0, p)
            try:
                import concourse  # noqa: F401

                return
            except ImportError:
                sys.path.pop(0)
        raise


_ensure_path()

import concourse.bass as bass  # noqa: E402
from concourse import bacc  # noqa: E402
import concourse.tile as tile  # noqa: E402
from concourse import mybir  # noqa: E402
from concourse import masks  # noqa: E402
from concourse.vector_clock import ScopedClock, VectorClock  # noqa: E402


# --- workaround: this walrus build rejects >1 sync-wait on the TileContext
# kernel-tail drain ("Too many sync wait commands", CoreV3GenImpl.cpp:104).
# Emit one drain per busy proc, each carrying a single sem wait.
def _patched_drain_and_barrier(self, tick_clock, wait_clock):
    gc = tick_clock.global_clock
    n = len(gc)
    for p in range(n):
        t = gc[p]
        if t <= 0:
            continue
        vec = [0] * n
        vec[p] = t
        d = self.nc.sync.drain()
        wait_clock.add_sem_waits(d.ins, ScopedClock({None: VectorClock(vec)}))

    self.nc.all_engine_barrier()
    assert self.sems is not None
    popped = self.nc._tile_sem_poison_stack.pop()
    assert popped is self._sem_poison
    self.nc.clear_and_free_semaphores(list(self.sems.allocated().values()))
    self.nc.all_engine_barrier()


tile.TileContext._drain_and_barrier = _patched_drain_and_barrier

B, T, H, R = 4, 2048, 768, 64
HC = H // 128  # 6 h-chunks
SC = T // 128  # 16 s-chunks
NSB = T // 512  # 4 query superblocks
N_CORES = 4
SCALE = float(1.0 / np.sqrt(H))
FP32 = mybir.dt.float32
BF16 = mybir.dt.bfloat16
Exp = mybir.ActivationFunctionType.Exp
ALU = mybir.AluOpType

LAST_RESULTS = None


def _emit(tc, nc, xb, wp, mk, out):
    from contextlib import ExitStack

    with ExitStack() as ctx:
        p_xn = ctx.enter_context(tc.tile_pool(name="p_xn", bufs=1))
        p_xT = ctx.enter_context(tc.tile_pool(name="p_xT", bufs=1))
        p_q = ctx.enter_context(tc.tile_pool(name="p_q", bufs=1))
        p_v = ctx.enter_context(tc.tile_pool(name="p_v", bufs=1))
        p_att = ctx.enter_context(tc.tile_pool(name="p_att", bufs=1))
        p_w = ctx.enter_context(tc.tile_pool(name="p_w", bufs=1))
        p_u = ctx.enter_context(tc.tile_pool(name="p_u", bufs=1))
        p_o = ctx.enter_context(tc.tile_pool(name="p_o", bufs=3))
        p_r = ctx.enter_context(tc.tile_pool(name="p_r", bufs=4))

        # ---- DMAs (all rows-contiguous: this walrus build rejects
        # sync-waits on strided DIRECT2D pseudo-DMAs) ----
        aqT_sb = p_w.tile([R, H], BF16, name="aqT_sb")
        bq_sb = p_w.tile([R, H], BF16, name="bq_sb")
        avT_sb = p_w.tile([R, H], BF16, name="avT_sb")
        bv_sb = p_w.tile([R, H], BF16, name="bv_sb")
        nc.gpsimd.dma_start(out=aqT_sb[:, :], in_=wp[0:R, :])
        nc.gpsimd.dma_start(out=bq_sb[:, :], in_=wp[R : 2 * R, :])
        nc.gpsimd.dma_start(out=avT_sb[:, :], in_=wp[2 * R : 3 * R, :])
        nc.gpsimd.dma_start(out=bv_sb[:, :], in_=wp[3 * R : 4 * R, :])

        # bias[s] = (mask-1)*1e30, precomputed host-side, one [128,1] per s-chunk
        bias_t = [p_w.tile([128, 1], FP32, name=f"bias{j}") for j in range(SC)]
        for j in range(SC):
            nc.gpsimd.dma_start(out=bias_t[j][:, :], in_=mk[j : j + 1, :].rearrange("n p -> p n"))

        xn_sb = [p_xn.tile([128, H], BF16, name=f"xn{j}") for j in range(SC)]
        for j in range(SC):
            nc.gpsimd.dma_start(out=xn_sb[j][:, :], in_=xb[j * 128 : (j + 1) * 128, :])

        id_sb = p_w.tile([128, 128], BF16, name="id_sb")
        masks.make_identity(nc, id_sb[:, :])

        # ---- PE transposes: xn -> xT, and A^T rows -> A (lhsT layout) ----
        xT_sb = [p_xT.tile([128, T], BF16, name=f"xT{i}") for i in range(HC)]
        aq_sb = [p_w.tile([128, R], BF16, name=f"aq_sb{i}") for i in range(HC)]
        av_sb = [p_w.tile([128, R], BF16, name=f"av_sb{i}") for i in range(HC)]
        with tc.tile_pool(name="psT", bufs=4, space="PSUM") as psT:
            for i in range(HC):
                hs = slice(i * 128, (i + 1) * 128)
                pa = psT.tile([128, R], BF16, name="pa", tag="pst")
                nc.tensor.transpose(pa[:, :], aqT_sb[:, hs], id_sb[0:R, 0:R])
                nc.scalar.copy(aq_sb[i][:, :], pa[:, :])
                pv = psT.tile([128, R], BF16, name="pv", tag="pst")
                nc.tensor.transpose(pv[:, :], avT_sb[:, hs], id_sb[0:R, 0:R])
                nc.scalar.copy(av_sb[i][:, :], pv[:, :])
            for j in range(SC):
                for i in range(HC):
                    pt = psT.tile([128, 128], BF16, name="pt", tag="pst")
                    nc.tensor.transpose(
                        pt[:, :], xn_sb[j][:, i * 128 : (i + 1) * 128], id_sb[:, :]
                    )
                    nc.scalar.copy(
                        xT_sb[i][:, j * 128 : (j + 1) * 128], pt[:, :]
                    )

        q_sb = [p_q.tile([128, T], BF16, name=f"q{i}") for i in range(HC)]
        uq_sb = p_u.tile([R, T], BF16, name="uq_sb")
        uv_sb = p_u.tile([R, T], BF16, name="uv_sb")
        bq = bq_sb[:, :]
        bv = bv_sb[:, :]

        with tc.tile_pool(name="psL", bufs=2, space="PSUM") as psL:
            # uqT [64, T] = Aq^T @ xT ; uvT [64, T] = Av^T @ xT
            for tq in range(T // 512):
                ts = slice(tq * 512, (tq + 1) * 512)
                ps = psL.tile([R, 512], FP32, name="pslq", tag="psl")
                for i in range(HC):
                    nc.tensor.matmul(
                        ps[:, :],
                        lhsT=aq_sb[i][:, :],
                        rhs=xT_sb[i][:, ts],
                        start=(i == 0),
                        stop=(i == HC - 1),
                    )
                nc.scalar.copy(uq_sb[:, ts], ps[:, :])
                ps = psL.tile([R, 512], FP32, name="pslv", tag="psl")
                for i in range(HC):
                    nc.tensor.matmul(
                        ps[:, :],
                        lhsT=av_sb[i][:, :],
                        rhs=xT_sb[i][:, ts],
                        start=(i == 0),
                        stop=(i == HC - 1),
                    )
                nc.scalar.copy(uv_sb[:, ts], ps[:, :])
            # qT = xT + Bq^T @ uqT
            for i in range(HC):
                for tq in range(T // 512):
                    ts = slice(tq * 512, (tq + 1) * 512)
                    ps = psL.tile([128, 512], FP32, name="pslb", tag="psl")
                    nc.tensor.matmul(
                        ps[:, :],
                        lhsT=bq[:, i * 128 : (i + 1) * 128],
                        rhs=uq_sb[:, ts],
                        start=True,
                        stop=True,
                    )
                    nc.vector.tensor_add(q_sb[i][:, ts], ps[:, :], xT_sb[i][:, ts])
            # v[s, :768] = x[s, :] + (Bv^T @ uvT)^T ; v[s, 768] = 1.0
            v_sb = []
            for j in range(SC):
                vj = p_v.tile([128, 772], BF16, name=f"v{j}")
                nc.vector.memset(vj[:, 768:769], 1.0)
                ps = psL.tile([128, 768], FP32, name="pslc", tag="psl")
                nc.tensor.matmul(
                    ps[:, 0:512],
                    lhsT=uv_sb[:, j * 128 : (j + 1) * 128],
                    rhs=bv[:, 0:512],
                    start=True,
                    stop=True,
                )
                nc.tensor.matmul(
                    ps[:, 512:768],
                    lhsT=uv_sb[:, j * 128 : (j + 1) * 128],
                    rhs=bv[:, 512:768],
                    start=True,
                    stop=True,
                )
                nc.vector.tensor_add(vj[:, 0:768], ps[:, 0:768], xn_sb[j][:, :])
                v_sb.append(vj)

        # ---- attention: 4 superblocks of 512 query cols ----
        with (
            tc.tile_pool(name="ps_s", bufs=2, space="PSUM") as ps_s,
            tc.tile_pool(name="ps_o", bufs=2, space="PSUM") as ps_o,
        ):
            for SB in range(NSB):
                qs = slice(SB * 512, (SB + 1) * 512)
                att = []
                for j in range(SC):
                    ps = ps_s.tile([128, 512], FP32, name="pss", tag="pss")
                    for i in range(HC):
                        nc.tensor.matmul(
                            ps[:, :],
                            lhsT=xT_sb[i][:, j * 128 : (j + 1) * 128],
                            rhs=q_sb[i][:, qs],
                            start=(i == 0),
                            stop=(i == HC - 1),
                        )
                    attj = p_att.tile([128, 512], BF16, name=f"att{j}")
                    nc.scalar.activation(
                        attj[:, :], ps[:, :], Exp, bias=bias_t[j][:, :], scale=SCALE
                    )
                    att.append(attj)
                for c in range(4):
                    pso = ps_o.tile([128, 772], FP32, name="pso", tag="pso")
                    for j in range(SC):
                        nc.tensor.matmul(
                            pso[:, 0:512],
                            lhsT=att[j][:, c * 128 : (c + 1) * 128],
                            rhs=v_sb[j][:, 0:512],
                            start=(j == 0),
                            stop=(j == SC - 1),
                        )
                        nc.tensor.matmul(
                            pso[:, 512:769],
                            lhsT=att[j][:, c * 128 : (c + 1) * 128],
                            rhs=v_sb[j][:, 512:769],
                            start=(j == 0),
                            stop=(j == SC - 1),
                        )
                    tr = SB * 512 + c * 128
                    rc = p_r.tile([128, 1], FP32, name="rc")
                    nc.vector.reciprocal(rc[:, :], pso[:, 768:769])
                    ob = p_o.tile([128, H], BF16, name="ob")
                    nc.vector.tensor_scalar(
                        ob[:, :], pso[:, 0:768], rc[:, :], None, ALU.mult
                    )
                    nc.gpsimd.dma_start(out=out[tr : tr + 128, :], in_=ob[:, :])


_NC_CACHE = None


def _build_nc():
    global _NC_CACHE
    if _NC_CACHE is not None:
        return _NC_CACHE
    nc = bacc.Bacc("TRN2", target_bir_lowering=False, debug=False)
    xb = nc.dram_tensor("xb", [T, H], BF16, kind="ExternalInput").ap()
    wp = nc.dram_tensor("wp", [256, H], BF16, kind="ExternalInput").ap()
    mk = nc.dram_tensor("mk", [SC, 128], FP32, kind="ExternalInput").ap()
    out = nc.dram_tensor("out", [T, H], BF16, kind="ExternalOutput").ap()

    import os

    linearize = bool(int(os.environ.get("KERNEL_LINEARIZE", "0")))
    with tile.TileContext(nc, linearize=linearize) as tc:
        _emit(tc, nc, xb, wp, mk, out)
    nc.compile()
    _NC_CACHE = nc
    return nc


_RUNNER = None


def _build_runner():
    """Build the bass module once and wrap it in a CACHED
    jax.jit(shard_map(bass_exec)) callable plus an on-device zeros
    factory for the donated output buffers. Mirrors
    concourse.bass2jax.run_bass_via_pjrt, but hoists everything
    per-call-invariant out of the call path (run_bass_via_pjrt builds a
    fresh closure every call, so jax re-traces and re-compiles each
    time -- seconds per call)."""
    global _RUNNER
    if _RUNNER is not None:
        return _RUNNER

    nc = _build_nc()

    from concourse import bass2jax
    import jax
    import jax.numpy as jnp
    from jax.sharding import Mesh, PartitionSpec, NamedSharding
    from jax.experimental.shard_map import shard_map

    bass2jax.install_neuronx_cc_hook()
    assert nc.dbg_addr is None
    partition_name = nc.partition_id_tensor.name if nc.partition_id_tensor else None

    in_names, out_names, out_avals, zero_shapes = [], [], [], []
    for alloc in nc.m.functions[0].allocations:
        if not isinstance(alloc, mybir.MemoryLocationSet):
            continue
        name = alloc.memorylocations[0].name
        if alloc.kind == "ExternalInput":
            if name != partition_name:
                in_names.append(name)
        elif alloc.kind == "ExternalOutput":
            shape = tuple(alloc.tensor_shape)
            dtype = mybir.dt.np(alloc.dtype)
            out_names.append(name)
            out_avals.append(jax.core.ShapedArray(shape, dtype))
            zero_shapes.append((shape, dtype))
    n_params = len(in_names)
    n_outs = len(out_avals)
    all_in_names = list(in_names) + list(out_names)
    if partition_name is not None:
        all_in_names.append(partition_name)
    donate = tuple(range(n_params, n_params + n_outs))

    def _body(*args):
        operands = list(args)
        if partition_name is not None:
            operands.append(bass2jax.partition_id_tensor())
        outs = bass2jax._bass_exec_p.bind(
            *operands,
            out_avals=tuple(out_avals),
            in_names=tuple(all_in_names),
            out_names=tuple(out_names),
            lowering_input_output_aliases=(),
            sim_require_finite=True,
            sim_require_nnan=True,
            nc=nc,
        )
        return tuple(outs)

    devices = jax.devices()[:N_CORES]
    mesh = Mesh(np.asarray(devices), ("core",))
    in_specs = (PartitionSpec("core"),) * (n_params + n_outs)
    out_specs = (PartitionSpec("core"),) * n_outs
    sharded = jax.jit(
        shard_map(
            _body, mesh=mesh, in_specs=in_specs, out_specs=out_specs, check_rep=False
        ),
        donate_argnums=donate,
        keep_unused=True,
    )
    zshard = NamedSharding(mesh, PartitionSpec("core"))
    zeros_fn = jax.jit(
        lambda: tuple(
            jnp.zeros((N_CORES * s[0], *s[1:]), d) for (s, d) in zero_shapes
        ),
        out_shardings=(zshard,) * n_outs,
    )
    _RUNNER = dict(
        sharded=sharded,
        zeros_fn=zeros_fn,
        in_names=in_names,
        out_avals=out_avals,
    )
    return _RUNNER


def kernel(hidden_states, mask, A_q, B_q, A_v, B_v):
    import ml_dtypes

    bf16 = ml_dtypes.bfloat16
    r = _build_runner()
    # donated output buffers: created on-device (async dispatch), never
    # cross the wire
    zeros = r["zeros_fn"]()

    x = np.asarray(hidden_states)
    if x.dtype != np.float32:
        x = x.astype(np.float32)
    # [B*T, H] bf16 -- the only bulk host->device transfer (12.6 MB)
    xb = x.reshape(B * T, H).astype(bf16)

    wrow = np.concatenate(
        [
            np.ascontiguousarray(np.asarray(A_q, dtype=np.float32).T),
            np.asarray(B_q, dtype=np.float32),
            np.ascontiguousarray(np.asarray(A_v, dtype=np.float32).T),
            np.asarray(B_v, dtype=np.float32),
        ],
        axis=0,
    ).astype(bf16)  # [256, H]
    wp = np.tile(wrow, (N_CORES, 1))

    mkb = (
        (np.asarray(mask, dtype=np.float32).reshape(B * SC, 128) > 0).astype(np.float32)
        - 1.0
    ) * 1e30

    out_arrs = r["sharded"](xb, wp, mkb, *zeros)
    out = np.asarray(out_arrs[0]).astype(np.float32).reshape(B, T, H)
    return out
